# revision 2
# baseline (speedup 1.0000x reference)
"""Trainium2 Bass kernel for nn_CapLayerLP — 26-way multiway dual bisection, v7.

v3 over v2: bf16 predicate-count matmuls (fp32 matmuls lower to two
half-speed instructions), single shared bias state (ScalarE reads the
DVE-updated bias tile instead of keeping a private copy), rebalanced
engine split (XS=400), broadcast DMAs split in halves across four
queues, gpsimd iota issued before its DMA issues, and a shortened
post-FM select chain (branch deltas precomputed while FM is in flight).
"""
import numpy as np

import concourse.bass as bass
import concourse.bacc as bacc
import concourse.tile as tile
from concourse import mybir
from concourse.bass_utils import run_bass_kernel_spmd

AL = mybir.AluOpType
F32 = mybir.dt.float32
BF16 = mybir.dt.bfloat16
I32 = mybir.dt.int32
AF = mybir.ActivationFunctionType
AX = mybir.AxisListType.X

N = 1024
P = 128
CO = N // P            # 8
K = 25                 # candidates per lane
NSTEP = 4
XS = 400               # ScalarE columns; DVE gets N-XS
NH = 512               # DMA half width
C_CAP = 10.0
Q = C_CAP / N
SCALE = 1e4            # 1/eps
SGV = 6.5536           # 65536/SCALE source mask offset
LOFFV = -65536.0
H = [4.0 / 26.0 ** (k + 1) for k in range(NSTEP)]


def _build(nc: bass.Bass):
    x_d = nc.dram_tensor("x", [1, N], F32, kind="ExternalInput")
    f_d = nc.dram_tensor("ind", [N], I32, kind="ExternalInput")
    out_d = nc.dram_tensor("out", [1, N], F32, kind="ExternalOutput")

    x_row = x_d[:, :]                                    # [1, N]
    f_row = f_d[:].rearrange("(a n) -> a n", a=1)        # [1, N]
    x2d = x_d[:, :].rearrange("a (p c) -> a p c", p=P)[0]
    f2d = f_d[:].rearrange("(p c) -> p c", p=P)
    o2d = out_d[:, :].rearrange("a (p c) -> a p c", p=P)[0]

    with tile.TileContext(nc) as tc:
        with (
            tc.tile_pool(name="const", bufs=1) as cns,
            tc.tile_pool(name="state", bufs=1) as st,
            tc.tile_pool(name="scr", bufs=2) as sc,
            tc.tile_pool(name="psum", bufs=1, space="PSUM") as ps,
            tc.tile_pool(name="psum2", bufs=2, space="PSUM") as ps2,
        ):
            v = nc.vector
            g = nc.gpsimd
            a = nc.scalar
            pe = nc.tensor

            # ---- constants that gate nothing else, issued first ----
            PF1 = cns.tile([P, 1], F32)      # p + 1 (f32 iota)
            g.iota(PF1, pattern=[[0, 1]], base=1, channel_multiplier=1,
                   allow_small_or_imprecise_dtypes=True)

            # ---- DMAs: x/f broadcast halves on four queues ----
            XBR = st.tile([P, N], F32)       # x broadcast to all partitions
            FBR = st.tile([P, N], I32)       # f broadcast (raw int32, no cast)
            C1, C2 = 512, 768                # chunk edges (c1 covers XS cols)
            nc.sync.dma_start(out=XBR[:, 0:C1],
                              in_=x_row[:, 0:C1].broadcast_to([P, C1]))
            a.dma_start(out=FBR[:, 0:C1],
                        in_=f_row[:, 0:C1].broadcast_to([P, C1]))
            F8 = cns.tile([P, CO], F32)
            g.dma_start(out=F8, in_=f2d)     # only cast DMA (4 KB, off path)
            X2d = cns.tile([P, CO], F32)
            g.dma_start(out=X2d, in_=x2d)
            g.dma_start(out=XBR[:, C1:C2],
                        in_=x_row[:, C1:C2].broadcast_to([P, C2 - C1]))
            nc.sync.dma_start(out=FBR[:, C2:N],
                              in_=f_row[:, C2:N].broadcast_to([P, N - C2]))
            a.dma_start(out=XBR[:, C2:N],
                        in_=x_row[:, C2:N].broadcast_to([P, N - C2]))
            g.dma_start(out=FBR[:, C1:C2],
                        in_=f_row[:, C1:C2].broadcast_to([P, C2 - C1]))

            # ---- constants (hidden under the DMAs) ----
            # Engine ops must start at partition 0/32/64/96, so the
            # lane-structured [P,1] constants (lane blocks of 25) are built
            # arithmetically from a full-partition iota instead of memsets.
            ONES = cns.tile([P, P], F32)
            v.memset(ONES, 1.0)
            TH = cns.tile([P, 5], F32)       # t_j = (p >= 25j), j=1..5
            for j in range(1, 6):
                v.tensor_scalar(out=TH[:, j - 1:j], in0=PF1,
                                scalar1=25.0 * j, scalar2=None,
                                op0=AL.is_gt)
            LF = cns.tile([P, 1], F32)       # lane id = sum of thresholds
            s12 = sc.tile([P, 1], F32, tag="c_s12")
            v.tensor_tensor(out=s12, in0=TH[:, 0:1], in1=TH[:, 1:2],
                            op=AL.add)
            s34 = sc.tile([P, 1], F32, tag="c_s34")
            v.tensor_tensor(out=s34, in0=TH[:, 2:3], in1=TH[:, 3:4],
                            op=AL.add)
            s1234 = sc.tile([P, 1], F32, tag="c_s1234")
            v.tensor_tensor(out=s1234, in0=s12, in1=s34, op=AL.add)
            v.tensor_tensor(out=LF, in0=s1234, in1=TH[:, 4:5], op=AL.add)
            # u = t1 - 2*t3 -> {0 lane0, +1 lanes 1-2, -1 lanes 3-4+}
            UPM = cns.tile([P, 1], F32)
            v.scalar_tensor_tensor(out=UPM, in0=TH[:, 2:3], scalar=-2.0,
                                   in1=TH[:, 0:1], op0=AL.mult, op1=AL.add)
            SGN = cns.tile([P, 1], F32)      # source mask sign
            v.tensor_scalar(out=SGN, in0=UPM, scalar1=SGV, scalar2=None,
                            op0=AL.mult)
            BV = cns.tile([P, 1], F32)       # target Nm-coefficient
            v.tensor_scalar(out=BV, in0=UPM, scalar1=Q, scalar2=None,
                            op0=AL.mult)
            LOFF = cns.tile([P, 1], F32)     # male-lane bias offset
            um = sc.tile([P, 1], F32, tag="c_um")
            v.tensor_tensor(out=um, in0=TH[:, 0:1], in1=TH[:, 2:3],
                            op=AL.subtract)
            v.tensor_scalar(out=LOFF, in0=um, scalar1=LOFFV, scalar2=None,
                            op0=AL.mult)
            # A = 10 - 10*t1 + t2 + 9*t3 - t4 + t5 ; AV2 = A - XS
            AV2 = cns.tile([P, 1], F32)      # target const part - XS
            a1 = sc.tile([P, 1], F32, tag="c_a1")
            v.scalar_tensor_tensor(out=a1, in0=TH[:, 0:1], scalar=-10.0,
                                   in1=TH[:, 1:2], op0=AL.mult, op1=AL.add)
            a2 = sc.tile([P, 1], F32, tag="c_a2")
            v.scalar_tensor_tensor(out=a2, in0=TH[:, 2:3], scalar=9.0,
                                   in1=a1, op0=AL.mult, op1=AL.add)
            a3 = sc.tile([P, 1], F32, tag="c_a3")
            v.tensor_tensor(out=a3, in0=a2, in1=TH[:, 3:4], op=AL.subtract)
            a4 = sc.tile([P, 1], F32, tag="c_a4")
            v.tensor_tensor(out=a4, in0=a3, in1=TH[:, 4:5], op=AL.add)
            v.tensor_scalar(out=AV2, in0=a4, scalar1=C_CAP - XS,
                            scalar2=None, op0=AL.add)
            SEL5 = cns.tile([P, 5], F32)     # one-hot: p == 25j
            g.affine_select(out=SEL5, in_=ONES[:, 0:5], pattern=[[-25, 5]],
                            compare_op=AL.is_equal, fill=0.0,
                            base=0, channel_multiplier=1)
            CP1 = cns.tile([P, 1], F32)      # candidate idx + 1
            v.scalar_tensor_tensor(out=CP1, in0=LF, scalar=-25.0, in1=PF1,
                                   op0=AL.mult, op1=AL.add)
            NT1 = cns.tile([P, 1], F32)      # step-1 bias: -1e4*h1*(c+1)+loff
            v.scalar_tensor_tensor(out=NT1, in0=CP1, scalar=-SCALE * H[0],
                                   in1=LOFF, op0=AL.mult, op1=AL.add)
            DK = cns.tile([P, 5], F32)       # per-step bias increments
            for k in range(NSTEP - 1):
                v.tensor_scalar(out=DK[:, k:k + 1], in0=CP1,
                                scalar1=SCALE * (H[k] - H[k + 1]),
                                scalar2=None, op0=AL.mult)
            MH = sc.tile([P, 1], F32, tag="mh")
            v.tensor_scalar(out=MH, in0=LOFF, scalar1=-1.0,
                            scalar2=-SCALE * H[NSTEP - 1] / 2.0,
                            op0=AL.mult, op1=AL.add)
            v.scalar_tensor_tensor(out=DK[:, NSTEP - 1:NSTEP], in0=CP1,
                                   scalar=SCALE * H[NSTEP - 1], in1=MH,
                                   op0=AL.mult, op1=AL.add)

            ONESB = cns.tile([P, P], BF16)
            v.memset(ONESB, 1.0)
            J5 = cns.tile([P, 5], F32)       # 0..4 along free dim
            g.iota(J5, pattern=[[1, 5]], base=0, channel_multiplier=0,
                   allow_small_or_imprecise_dtypes=True)
            LFB5 = sc.tile([P, 5], F32, tag="lfb5")
            v.tensor_scalar(out=LFB5, in0=ONES[:, 0:5], scalar1=LF[:, 0:1],
                            scalar2=None, op0=AL.mult)
            LONEHOT = cns.tile([P, 5], BF16)  # lane one-hot along free dim
            v.tensor_tensor(out=LONEHOT, in0=LFB5, in1=J5, op=AL.is_equal)

            # lane-block 0/1 matrix (bf16) for per-lane predicate counting
            LBC = cns.tile([P, P], F32)
            v.tensor_scalar(out=LBC, in0=ONES, scalar1=LF[:, 0:1],
                            scalar2=None, op0=AL.mult)
            DG = cns.tile([P, P], F32)
            g.affine_select(out=DG, in_=LBC, pattern=[[1, P]],
                            compare_op=AL.is_equal, fill=0.0,
                            base=0, channel_multiplier=-1)
            LRp = ps.tile([P, P], F32, tag="lr")
            pe.matmul(LRp, ONES, DG)
            W2 = cns.tile([P, P], BF16)
            v.tensor_tensor(out=W2, in0=LBC, in1=LRp, op=AL.is_equal)

            # targets (need Nm = sum f)
            NM8 = sc.tile([P, 1], F32, tag="nm8")
            v.reduce_sum(NM8, F8, axis=AX)
            NMp = ps.tile([P, 1], F32, tag="nmp")
            pe.matmul(NMp, ONES, NM8)
            TGT2 = cns.tile([P, 1], F32)
            v.scalar_tensor_tensor(out=TGT2, in0=NMp, scalar=BV[:, 0:1],
                                   in1=AV2, op0=AL.mult, op1=AL.add)
            BL2 = cns.tile([P, 1], F32)
            v.tensor_scalar(out=BL2, in0=NMp, scalar1=Q, scalar2=None,
                            op0=AL.mult)
            BH2 = cns.tile([P, 1], F32)
            v.tensor_scalar(out=BH2, in0=BL2, scalar1=1.0, scalar2=None,
                            op0=AL.add)

            # finale 2D sources
            XM2 = cns.tile([P, CO], F32)
            v.tensor_tensor(out=XM2, in0=X2d, in1=F8, op=AL.mult)
            XF2 = cns.tile([P, CO], F32)
            v.tensor_tensor(out=XF2, in0=X2d, in1=XM2, op=AL.subtract)
            ONEW = cns.tile([P, N - XS], F32)   # ones for clip-top via stt
            v.memset(ONEW, 1.0)

            # shared source: x + sgn(p)*6.5536*f  (three chunks, pipelined)
            SRC = st.tile([P, N], F32)
            for lo, hi in ((0, C1), (C1, C2), (C2, N)):
                v.scalar_tensor_tensor(out=SRC[:, lo:hi],
                                       in0=FBR[:, lo:hi],
                                       scalar=SGN[:, 0:1],
                                       in1=XBR[:, lo:hi],
                                       op0=AL.mult, op1=AL.add)

            # ---- 5 multiway steps (shared bias state NTD) ----
            NTD = NT1
            for k in range(NSTEP):
                hk = H[k]
                U = sc.tile([P, XS], F32, tag="U")
                a.activation(U, SRC[:, 0:XS], AF.Relu,
                             bias=NTD[:, 0:1], scale=SCALE)
                J = sc.tile([P, XS], F32, tag="J")
                ACCS = sc.tile([P, 1], F32, tag="accs")
                a.activation(J, U, AF.Relu, bias=1.0, scale=-1.0,
                             accum_out=ACCS)
                T = sc.tile([P, N - XS], F32, tag="T")
                v.tensor_scalar(out=T, in0=SRC[:, XS:], scalar1=SCALE,
                                scalar2=NTD[:, 0:1], op0=AL.mult,
                                op1=AL.add)
                CL = sc.tile([P, N - XS], F32, tag="CL")
                ACCV = sc.tile([P, 1], F32, tag="accv")
                v.scalar_tensor_tensor(out=CL, in0=T, scalar=0.0,
                                       in1=ONEW, op0=AL.max, op1=AL.min,
                                       accum_out=ACCV)
                last = (k == NSTEP - 1)
                if last:
                    # pre-count bias lane-broadcast, hidden under the big ops
                    NTDp = sc.tile([P, 1], F32, tag="ntdp")
                    v.tensor_tensor(out=NTDp, in0=NTD,
                                    in1=DK[:, k:k + 1], op=AL.add)
                    B5a = sc.tile([P, 5], F32, tag="b5a")
                    v.tensor_scalar(out=B5a, in0=SEL5,
                                    scalar1=NTDp[:, 0:1], scalar2=None,
                                    op0=AL.mult)
                    T5p = ps.tile([P, 5], F32, tag="t5")
                    pe.matmul(T5p, ONES, B5a)
                    T5A = sc.tile([P, 5], F32, tag="t5a")
                    v.tensor_scalar(out=T5A, in0=T5p, scalar1=0.0,
                                    scalar2=None, op0=AL.add)
                PRED = sc.tile([P, 1], F32 if k == NSTEP - 1 else BF16,
                               tag="pred")
                v.scalar_tensor_tensor(out=PRED, in0=ACCV,
                                       scalar=TGT2[:, 0:1], in1=ACCS,
                                       op0=AL.subtract, op1=AL.is_gt)
                if not last:
                    CNT = ps2.tile([P, 1], F32, tag="cnt")
                    pe.matmul(CNT, W2, PRED)
                    NTDp = sc.tile([P, 1], F32, tag="ntdp")
                    v.tensor_tensor(out=NTDp, in0=NTD,
                                    in1=DK[:, k:k + 1], op=AL.add)
                    NTD2 = sc.tile([P, 1], F32, tag="ntd")
                    v.scalar_tensor_tensor(out=NTD2, in0=CNT,
                                           scalar=-SCALE * hk, in1=NTDp,
                                           op0=AL.mult, op1=AL.add)
                    NTD = NTD2
                else:
                    # per-lane counts broadcast along the free dim, then
                    # TAU5 = T5A - 1e4*h*cnt directly (one bf16 matmul)
                    PL = sc.tile([P, 5], BF16, tag="pl")
                    v.tensor_scalar(out=PL, in0=LONEHOT,
                                    scalar1=PRED[:, 0:1], scalar2=None,
                                    op0=AL.mult)
                    CNTB = ps2.tile([P, 5], F32, tag="cntb")
                    pe.matmul(CNTB, ONESB, PL)
                    TAU5 = st.tile([P, 5], F32)
                    v.scalar_tensor_tensor(out=TAU5, in0=CNTB,
                                           scalar=-SCALE * hk, in1=T5A,
                                           op0=AL.mult, op1=AL.add)

            # ---- finale: branch select in 2D layout ----

            XAr = sc.tile([P, CO], F32, tag="xar")
            a.activation(XAr, X2d, AF.Relu, bias=TAU5[:, 0:1], scale=SCALE)
            fmv = sc.tile([P, CO], F32, tag="fmv")
            AFm = sc.tile([P, 1], F32, tag="afm")
            v.scalar_tensor_tensor(out=fmv, in0=XAr, scalar=1.0, in1=F8,
                                   op0=AL.min, op1=AL.mult, accum_out=AFm)
            FM = ps.tile([P, 1], F32, tag="fm")
            pe.matmul(FM, ONES, AFm)
            XOUT = st.tile([P, CO], F32)     # branch-A output (clip top)
            v.tensor_scalar(out=XOUT, in0=XAr, scalar1=1.0, scalar2=None,
                            op0=AL.min)

            # 4 branch-B candidates: lanes 1-2 via ScalarE relu + DVE min,
            # lanes 3-4 fully on DVE (mult-add then max/min) to balance
            XC = [None] * 4
            for i, ln in ((0, 1), (1, 2)):
                xcr = sc.tile([P, CO], F32, tag=f"xcr{i}")
                a.activation(xcr, XM2, AF.Relu, bias=TAU5[:, ln:ln + 1],
                             scale=SCALE)
                xc = sc.tile([P, CO], F32, tag=f"xc{i}")
                v.tensor_scalar(out=xc, in0=xcr, scalar1=1.0, scalar2=None,
                                op0=AL.min)
                XC[i] = xc
            for i, ln in ((2, 3), (3, 4)):
                tc_ = sc.tile([P, CO], F32, tag=f"tc{i}")
                v.tensor_scalar(out=tc_, in0=XF2, scalar1=SCALE,
                                scalar2=TAU5[:, ln:ln + 1], op0=AL.mult,
                                op1=AL.add)
                xc = sc.tile([P, CO], F32, tag=f"xc{i}")
                v.tensor_scalar(out=xc, in0=tc_, scalar1=0.0, scalar2=1.0,
                                op0=AL.max, op1=AL.min)
                XC[i] = xc
            XBH = sc.tile([P, CO], F32, tag="xbh")
            v.tensor_tensor(out=XBH, in0=XC[1], in1=XC[3], op=AL.add)
            XBL = sc.tile([P, CO], F32, tag="xbl")
            v.tensor_tensor(out=XBL, in0=XC[0], in1=XC[2], op=AL.add)

            PREDH = sc.tile([P, 1], mybir.dt.uint8, tag="predh")
            v.tensor_tensor(out=PREDH, in0=FM, in1=BH2, op=AL.is_gt)
            PREDL = sc.tile([P, 1], mybir.dt.uint8, tag="predl")
            v.tensor_tensor(out=PREDL, in0=FM, in1=BL2, op=AL.is_lt)
            v.copy_predicated(XOUT, PREDH[:, 0:1].broadcast_to([P, CO]),
                              XBH)
            v.copy_predicated(XOUT, PREDL[:, 0:1].broadcast_to([P, CO]),
                              XBL)
            nc.sync.dma_start(out=o2d, in_=XOUT)

    return nc


_CACHE: dict = {}


def _get_nc():
    if "nc" not in _CACHE:
        nc = bacc.Bacc(None, target_bir_lowering=False)
        _build(nc)
        nc.finalize()
        _CACHE["nc"] = nc
    return _CACHE["nc"]


def kernel(x: np.ndarray, indices_male: np.ndarray) -> np.ndarray:
    nc = _get_nc()
    base = {
        "x": np.ascontiguousarray(x, dtype=np.float32),
        "ind": np.ascontiguousarray(indices_male, dtype=np.int32),
    }
    in_maps = [dict(base) for _ in range(8)]
    res = run_bass_kernel_spmd(nc, in_maps, core_ids=list(range(8)))
    return np.asarray(res.results[0]["out"], dtype=np.float32)


if __name__ == "__main__":
    rng = np.random.default_rng(0)
    x = rng.standard_normal((1, N)).astype(np.float32)
    f = (np.arange(N) % 2).astype(np.int32)
    out = kernel(x, f)
    print("out", out.shape, out.dtype, out[0, :6], out.sum())
